# revision 17
# baseline (speedup 1.0000x reference)
"""Sliding-window attention (L=4096, H=2048, 16 heads, window 1024) on 8 TRN2 cores.

Sequence sharding with pair-shared-HBM K/V exchange: core c owns query rows
[512c, 512c+512). HBM-pair cores (2p, 2p+1) cooperate: each core computes
K/V projections for only 1024 rows — the first 512 rows of its own window
([512c-1024, 512c-512)) plus its own query rows ([512c, 512c+512)) — and
writes them (RoPE'd K^T and natural V, bf16) to a pair-shared HBM buffer.
The middle 512 window rows come from the partner core. A tiny per-group
pair AllReduce acts as the write->read barrier (ordered with explicit dep
edges). Heads are processed in 4 groups of 4: projections of group g+1
overlap the barrier and attention of group g.

Engine split: PE does matmuls only; ACT does exps + output copies; DVE does
PSUM->SBUF copies, RoPE multiplies (PSUM-direct) and normalization; Pool
does the RoPE adds and the shared-HBM SWDGE traffic. Softmax exps are
batched per window k-tile. All matmuls run in bf16 (fp32 PSUM).
"""

import sys

import numpy as np

if "/opt/trn_rl_repo" not in sys.path:
    sys.path.insert(0, "/opt/trn_rl_repo")

L = 4096
H = 2048
NH = 16
D = 128
WIN = 1024
NCORES = 8
QROWS = L // NCORES          # 512 query rows per core
WROWS = QROWS + WIN          # 1536 window rows per core
CROWS = 2 * QROWS            # 1024 rows of K/V computed per core
NQT = QROWS // 128           # 4 q tiles per core
NWT = WROWS // 128           # 12 window k tiles per core
NKT = 9                      # k tiles attended per q tile
GH = 2                       # heads per barrier group
NG = NH // GH                # 4 groups
ROPE_THETA = 10000.0
SCALE = float(D) ** -0.5
NEG = -1e30

# per window k-tile kt: valid q tiles are lq in [kt-8, kt] ∩ [0, NQT)
_LQ0 = [max(0, kt - (NKT - 1)) for kt in range(NWT)]
_NV = [min(NQT - 1, kt) - _LQ0[kt] + 1 for kt in range(NWT)]
_OFF = [sum(_NV[:kt]) for kt in range(NWT)]
NPB = sum(_NV)               # 36 p-tile blocks per head

PAIRS = [[2 * p, 2 * p + 1] for p in range(NCORES // 2)]

_CACHE = {}


def _trace(tc, aps):
    from contextlib import ExitStack

    import concourse.bass as bass
    from concourse import mybir
    from concourse.tile_rust import add_dep_helper

    nc = tc.nc
    f32 = mybir.dt.float32
    i32 = mybir.dt.int32
    bf16 = mybir.dt.bfloat16
    AF = mybir.ActivationFunctionType
    ds = bass.ds
    hswT, wq, wk, wv, wo, cosw, sinw, tsel2, maskl, maskd, idb, out = aps

    ctx = ExitStack()
    const = ctx.enter_context(tc.tile_pool(name="const", bufs=1))
    hstp = ctx.enter_context(tc.tile_pool(name="hst", bufs=1))
    otp = ctx.enter_context(tc.tile_pool(name="otp", bufs=1))
    qrp = ctx.enter_context(tc.tile_pool(name="qrp", bufs=1))
    wstr = ctx.enter_context(tc.tile_pool(name="wstr", bufs=2))
    kvw = ctx.enter_context(tc.tile_pool(name="kvw", bufs=2))
    kvp = ctx.enter_context(tc.tile_pool(name="kvp", bufs=5))
    rope = ctx.enter_context(tc.tile_pool(name="rope", bufs=6))
    attn = ctx.enter_context(tc.tile_pool(name="attn", bufs=4))
    pbp = ctx.enter_context(tc.tile_pool(name="pbp", bufs=3))
    phc = ctx.enter_context(tc.tile_pool(name="phc", bufs=2))
    dram = ctx.enter_context(tc.tile_pool(name="dram", bufs=NG, space="DRAM"))
    ps_b = ctx.enter_context(tc.tile_pool(name="ps_b", bufs=2, space="PSUM"))
    actx = ExitStack()  # pre-o_proj PSUM pools
    ps_st = actx.enter_context(tc.tile_pool(name="ps_st", bufs=2, space="PSUM"))
    ps_v = actx.enter_context(tc.tile_pool(name="ps_v", bufs=2, space="PSUM"))
    ps_o = actx.enter_context(tc.tile_pool(name="ps_o", bufs=1, space="PSUM"))
    ps_t = actx.enter_context(tc.tile_pool(name="ps_t", bufs=1, space="PSUM"))

    # pair-shared K/V store, one tensor per head (the sim requires a single
    # writer instruction per Shared tensor): [128, kv, Q, jt, r] where Q
    # indexes the four 512-row quarters of the pair's 2048-row union.
    # This core writes quarters {par, 2+par} (= its chunk0 and its own q
    # rows) in ONE strided DMA and reads quarters [par, par+3).
    skv = [
        dram.tile([128, 2, 4, 4, 128], bf16, name=f"skv{h}",
                  addr_space="Shared")
        for h in range(NH)
    ]

    # ---- first-matmul critical path: wk(h0) on the sync queue before
    # anything else (the scalar queue is blocked ~1.3us by LoadActFuncSet,
    # and constants are not needed until several us in) ----
    wk0_b = wstr.tile([128, 16, 128], bf16, tag="wk_h")
    nc.sync.dma_start(out=wk0_b, in_=wk[:, 0, :, :])

    # ---- constants (cos/sin on the idle gpsimd queue; the rest are
    # needed late and go after the first hs chunk) ----
    cos_sb = const.tile([128, CROWS], bf16, name="cos_sb")
    nc.gpsimd.dma_start(out=cos_sb, in_=cosw)
    sin_sb = const.tile([128, CROWS], bf16, name="sin_sb")
    nc.gpsimd.dma_start(out=sin_sb, in_=sinw)
    maskl_sb = const.tile([128, 128], bf16, name="maskl_sb")
    maskd_sb = const.tile([128, 128], bf16, name="maskd_sb")
    idb_sb = const.tile([128, 128], bf16, name="idb_sb")
    tsel_sb = const.tile([128, NWT], f32, name="tsel_sb")

    par = nc.gpsimd.partition_id() % 2      # Pool-engine register (writes)
    par_sp = nc.sync.partition_id() % 2     # SP-engine register (reads)

    # attention outputs, transposed: [feat-part, kt(=head), q-tile, row]
    ot_sb = otp.tile([128, 16, NQT, 128], bf16, name="ot_sb")
    # all heads' RoPE'd q, persisted through the attention phase
    qr_all = qrp.tile([128, NH, NQT, 128], bf16, name="qr_all")

    def load_w(h, engs=None):
        whs = []
        srcs = ((wq, "wq_h"), (wk, "wk_h"), (wv, "wv_h"))
        engs = engs or (nc.sync, nc.sync, nc.sync)
        for (w_dram, wtag), eng in zip(srcs, engs):
            w_b = wstr.tile([128, 16, 128], bf16, tag=wtag)
            eng.dma_start(out=w_b, in_=w_dram[:, h, :, :])
            whs.append(w_b)
        return whs

    # pre-transposed hs rows this core projects: [H, 1024] =
    # rows [512c-1024, 512c-512) ++ [512c, 512c+512)
    hsT = hstp.tile([128, 16, CROWS], bf16, name="hsT")

    def load_hs_chunk(rb):
        for kt in range(16):
            eng = nc.sync if kt % 2 == 0 else nc.scalar
            eng.dma_start(
                out=hsT[:, kt, rb * 512:(rb + 1) * 512],
                in_=hswT[kt * 128:(kt + 1) * 128, rb * 512:(rb + 1) * 512],
            )

    load_hs_chunk(0)
    nc.sync.dma_start(out=maskl_sb, in_=maskl)
    nc.sync.dma_start(out=maskd_sb, in_=maskd)
    nc.sync.dma_start(out=idb_sb, in_=idb)
    nc.sync.dma_start(out=tsel_sb, in_=tsel2)
    wq0_b = wstr.tile([128, 16, 128], bf16, tag="wq_h")
    nc.sync.dma_start(out=wq0_b, in_=wq[:, 0, :, :])
    wv0_b = wstr.tile([128, 16, 128], bf16, tag="wv_h")
    nc.scalar.dma_start(out=wv0_b, in_=wv[:, 0, :, :])
    w_next = [wq0_b, wk0_b, wv0_b]
    load_hs_chunk(1)

    def rope_pair(dst, src_ps, c0):
        """RoPE: dst[d, r] = src[d, r]*cos[d, c0+r] + src[(d+64)%128, r]*sin[d, c0+r].
        dst/src are [128, 512]; sin carries the sign for the lower half.
        The half-rotated term reads PSUM directly (mixed-base-partition SBUF
        reads are rejected by the walrus verifier; PSUM+SBUF is allowed)."""
        cols = slice(c0, c0 + 512)
        t1 = rope.tile([128, 512], bf16, tag="t1")
        nc.vector.tensor_mul(t1, src_ps, cos_sb[:, cols])
        t2 = rope.tile([128, 512], bf16, tag="t2")
        nc.vector.tensor_mul(t2[0:64, :], src_ps[64:128, :], sin_sb[0:64, cols])
        nc.vector.tensor_mul(t2[64:128, :], src_ps[0:64, :], sin_sb[64:128, cols])
        nc.gpsimd.tensor_add(dst, t1, t2)

    def emit_proj(h):
        """K/V for this core's 1024 rows + Q; K/V go to pair-shared HBM.
        Returns the shared-write DMA instruction."""
        nonlocal w_next
        wq_h, wk_h, wv_h = w_next
        if h + 1 < NH:
            w_next = load_w(h + 1)

        # staging: [p, kv, chunk, jt, r]; K^T tiles at kv=0, V at kv=1
        kvo = kvw.tile([128, 2, 2, 4, 128], bf16, tag="kvo")

        def k_chunk(rb):
            ps = ps_b.tile([128, 512], f32, tag="b", name=f"kp{h}_{rb}")
            for kt in range(16):
                nc.tensor.matmul(
                    ps,
                    lhsT=wk_h[:, kt, :],
                    rhs=hsT[:, kt, rb * 512:(rb + 1) * 512],
                    start=(kt == 0),
                    stop=(kt == 15),
                )
            dst = kvo[:, 0, rb, :, :].rearrange("p a b -> p (a b)")
            rope_pair(dst, ps, rb * 512)

        def v_tiles(j0, j1):
            for jt in range(j0, j1):
                ps = ps_v.tile([128, 128], f32, tag="v", name=f"vp{h}_{jt}")
                for kt in range(16):
                    nc.tensor.matmul(
                        ps,
                        lhsT=hsT[:, kt, jt * 128:(jt + 1) * 128],
                        rhs=wv_h[:, kt, :],
                        start=(kt == 0),
                        stop=(kt == 15),
                    )
                nc.vector.tensor_copy(kvo[:, 1, jt // 4, jt % 4, :], ps)

        # chunk-0 work first so head 0 isn't blocked on the chunk-1 loads
        k_chunk(0)
        v_tiles(0, 4)
        k_chunk(1)
        v_tiles(4, 8)

        # Q for own rows (= computed rows [512, 1024)), RoPE'd
        ps = ps_b.tile([128, 512], f32, tag="b", name=f"qp{h}")
        for kt in range(16):
            nc.tensor.matmul(
                ps,
                lhsT=wq_h[:, kt, :],
                rhs=hsT[:, kt, 512:1024],
                start=(kt == 0),
                stop=(kt == 15),
            )
        rope_pair(
            qr_all[:, h, :, :].rearrange("p a b -> p (a b)"), ps, 512)

        # single strided write: quarters par and 2+par of both K and V
        wr = nc.gpsimd.dma_start(
            out=skv[h][:, :, ds(par, 2, 2), :, :], in_=kvo)
        return [wr]

    prev = None  # (h-1)'s (p_sb, v_h) for the delayed PV stage

    def emit_pv(h, lq, p_sb, v_h):
        o_ps = ps_o.tile([128, 132], f32, tag="o", name=f"o{h}_{lq}")
        for t in range(NKT):
            kt = lq + t
            slot = lq - _LQ0[kt]
            nc.tensor.matmul(
                o_ps[:, 0:129],
                lhsT=p_sb[:, _OFF[kt] + slot, :],
                rhs=v_h[:, kt, 0:129],
                start=(t == 0),
                stop=(t == NKT - 1),
            )
        rinv = attn.tile([128, 1], f32, tag="rinv")
        nc.vector.reciprocal(rinv, o_ps[:, 128:129])
        ao = attn.tile([128, 128], bf16, tag="ao")
        nc.vector.tensor_scalar_mul(ao, o_ps[:, 0:128], rinv)
        tp = ps_t.tile([128, 128], bf16, tag="t", name=f"aot{h}_{lq}")
        nc.tensor.transpose(tp, ao, idb_sb)
        nc.vector.tensor_copy(ot_sb[:, h, lq, :], tp)

    def emit_s_tile(h, kt, kr_h, p_sb, tail=False):
        lq0, nv = _LQ0[kt], _NV[kt]
        st = ps_st.tile([128, 512], f32, tag="st", name=f"st{h}_{kt}")
        # one wide matmul over all valid q-tiles (contiguous in qr_all);
        # window-edge/diagonal masks fold in as one extra accumulation
        # step (S += I @ M), keeping the S->exp chain PE->ACT only
        has_mask = kt < NQT or kt >= NKT - 1
        nc.tensor.matmul(
            st[:, 0:nv * 128],
            lhsT=kr_h[:, kt, :],
            rhs=qr_all[:, h, lq0:lq0 + nv, :].rearrange("p a b -> p (a b)"),
            start=True,
            stop=not has_mask,
        )
        if kt < NQT:  # left window edge tile for q-tile lq = kt
            c = (kt - lq0) * 128
            nc.tensor.matmul(
                st[:, c:c + 128], lhsT=idb_sb, rhs=maskl_sb,
                start=False, stop=True,
            )
        if kt >= NKT - 1:  # diagonal tile for q-tile lq = kt-8
            c = (kt - (NKT - 1) - lq0) * 128
            nc.tensor.matmul(
                st[:, c:c + 128], lhsT=idb_sb, rhs=maskd_sb,
                start=False, stop=True,
            )
        nc.scalar.activation(
            p_sb[:, _OFF[kt]:_OFF[kt] + nv, :]
            .rearrange("p a b -> p (a b)"),
            st[:, 0:nv * 128], AF.Exp,
            bias=tsel_sb[:, kt:kt + 1], scale=SCALE,
        )

    def emit_reads(h, cc):
        """Load the head's full-window K/V from pair-shared HBM (emitted
        right after the pair barrier so the reads overlap the next
        projection group)."""
        kr_h = kvp.tile([128, NWT, 128], bf16, tag="kr_h")
        v_h = kvp.tile([128, NWT, 130], bf16, tag="v_h")
        nc.vector.memset(v_h[:, :, 128:130], 0.0)
        nc.vector.memset(v_h[:, :, 128:129], 1.0)
        for g in range(3):
            rk = nc.sync.dma_start(
                out=kr_h[:, 4 * g:4 * g + 4, :],
                in_=skv[h][:, 0, ds(par_sp + g, 1), :, :])
            rv = nc.sync.dma_start(
                out=v_h[:, 4 * g:4 * g + 4, 0:128],
                in_=skv[h][:, 1, ds(par_sp + g, 1), :, :])
            if g == 1:  # partner-written quarter: needs the pair barrier.
                # g=0/2 read back this core's own writes (queue-ordered).
                add_dep_helper(rk.ins, cc.ins,
                               reason="kv read after pair barrier")
                add_dep_helper(rv.ins, cc.ins,
                               reason="kv read after pair barrier")
        return kr_h, v_h

    def emit_attn(h, kv, tail=False, filler=None):
        """S^T + exp for one head; (h-1)'s PV stages interleave as filler."""
        nonlocal prev
        kr_h, v_h = kv
        p_sb = pbp.tile([128, NPB, 128], bf16, tag="p_sb")
        for kt in range(NWT):
            emit_s_tile(h, kt, kr_h, p_sb, tail=tail)
            if prev is not None and kt % 3 == 2:
                emit_pv(h - 1, kt // 3, *prev)
            if filler is not None and kt % 2 == 1:
                next(filler, None)
        prev = (p_sb, v_h)

    def emit_barrier(wr_insts, g):
        arin = dram.tile([1, 4], i32, tag="arin", name=f"arin{g}")
        arout = dram.tile([1, 4], i32, tag="arout", name=f"arout{g}")
        cc = nc.gpsimd.collective_compute(
            "AllReduce",
            mybir.AluOpType.add,
            replica_groups=PAIRS,
            ins=[arin.opt()],
            outs=[arout.opt()],
        )
        for wr in wr_insts:
            add_dep_helper(cc.ins, wr.ins, reason="barrier after kv write")
        return cc

    # ---- schedule: proj(0), proj(1), attn(0), proj(2), attn(1),
    #                proj(3), attn(2), attn(3), o_proj ----
    ccs = [None] * NG

    kv_tiles = {}

    def proj_group(g):
        wrs = []
        for h in range(g * GH, (g + 1) * GH):
            wrs += emit_proj(h)
        ccs[g] = emit_barrier(wrs, g)
        for h in range(g * GH, (g + 1) * GH):
            kv_tiles[h] = emit_reads(h, ccs[g])

    def attn_group(g, tail=False, filler=None):
        for h in range(g * GH, (g + 1) * GH):
            emit_attn(h, kv_tiles.pop(h), tail=tail, filler=filler)

    proj_group(0)
    proj_group(1)
    proj_group(2)
    attn_group(0)
    proj_group(3)
    attn_group(1)
    proj_group(4)
    attn_group(2)
    proj_group(5)
    attn_group(3)
    proj_group(6)
    attn_group(4)
    proj_group(7)
    attn_group(5)

    # tail filler: accumulate o_proj output block nb=0 for q-tiles 0,1 in
    # the (idle after the last projection) ps_b banks, one kt-chunk per
    # fill point inside the work-starved last attention groups
    opT = [ps_b.tile([128, 512], f32, tag="b", name=f"opT{i}")
           for i in range(2)]

    def gen_opj_tail():
        for kt in range(12):
            wos = phc.tile([128, 512], bf16, tag="wos", bufs=10)
            (nc.sync if kt % 2 == 0 else nc.scalar).dma_start(
                out=wos, in_=wo[kt * 128:(kt + 1) * 128, 0:512])
            for i in range(2):
                nc.tensor.matmul(
                    opT[i], lhsT=ot_sb[:, kt, i, :], rhs=wos,
                    start=(kt == 0), stop=False,
                )
            yield

    _opf = gen_opj_tail()
    attn_group(6, filler=_opf)
    attn_group(7, filler=_opf)
    for lq in range(NQT):  # last head's PV
        emit_pv(NH - 1, lq, *prev)
    for _ in _opf:  # drain any unemitted tail chunks
        pass
    for kt in range(12, 16):  # finish the tail accumulators
        wos = phc.tile([128, 512], bf16, tag="wos", bufs=10)
        (nc.sync if kt % 2 == 0 else nc.scalar).dma_start(
            out=wos, in_=wo[kt * 128:(kt + 1) * 128, 0:512])
        for i in range(2):
            nc.tensor.matmul(
                opT[i], lhsT=ot_sb[:, kt, i, :], rhs=wos,
                start=False, stop=(kt == 15),
            )
    for i in range(2):
        ob = phc.tile([128, 512], f32, tag="ob", bufs=4)
        if i % 2 == 0:
            nc.scalar.copy(ob, opT[i])
        else:
            nc.vector.tensor_copy(ob, opT[i])
        (nc.sync if i % 2 == 0 else nc.scalar).dma_start(
            out=out[i, :, 0:512], in_=ob)

    # ---- o_proj: out[rows, :] = ot.T @ wo, streaming wo once (bf16).
    # All 4 q-tiles accumulate against each streamed wo tile.
    actx.close()
    ps_op = ctx.enter_context(tc.tile_pool(name="ps_op", bufs=1, space="PSUM"))
    for nb in range(4):
        irange = range(2, 4) if nb == 0 else range(4)
        pss = {
            i: ps_op.tile([128, 512], f32, tag=f"op{i}", name=f"op{nb}_{i}")
            for i in irange
        }
        for kt in range(16):
            wos = phc.tile([128, 512], bf16, tag="wos", bufs=10)
            (nc.sync if kt % 2 == 0 else nc.scalar).dma_start(
                out=wos,
                in_=wo[kt * 128:(kt + 1) * 128, nb * 512:(nb + 1) * 512],
            )
            for i in irange:
                nc.tensor.matmul(
                    pss[i], lhsT=ot_sb[:, kt, i, :], rhs=wos,
                    start=(kt == 0), stop=(kt == 15),
                )
        for i in irange:
            ob = phc.tile([128, 512], f32, tag="ob", bufs=4)
            if i % 2 == 0:
                nc.scalar.copy(ob, pss[i])
            else:
                nc.vector.tensor_copy(ob, pss[i])
            (nc.sync if i % 2 == 0 else nc.scalar).dma_start(
                out=out[i, :, nb * 512:(nb + 1) * 512], in_=ob
            )

    ctx.close()


def _build(timing=False):
    """Build the module. With timing=True, all real tensors become Internal
    DRAM (garbage contents, valid timing) and tiny dummy ExternalInput/Output
    tensors are added, so benchmarking excludes host<->device transfer."""
    import concourse.bacc as bacc
    import concourse.tile as tile
    from concourse import mybir

    f32 = mybir.dt.float32
    bf16 = mybir.dt.bfloat16

    nc = bacc.Bacc("TRN2", target_bir_lowering=False, debug=False,
                   num_devices=NCORES)
    kind = {} if timing else {"kind": "ExternalInput"}
    okind = {} if timing else {"kind": "ExternalOutput"}
    aps = [
        nc.dram_tensor("hswT", [H, CROWS], bf16, **kind).ap(),
        nc.dram_tensor("wq", [128, NH, 16, 128], bf16, **kind).ap(),
        nc.dram_tensor("wk", [128, NH, 16, 128], bf16, **kind).ap(),
        nc.dram_tensor("wv", [128, NH, 16, 128], bf16, **kind).ap(),
        nc.dram_tensor("wo", [H, H], bf16, **kind).ap(),
        nc.dram_tensor("cosw", [D, CROWS], bf16, **kind).ap(),
        nc.dram_tensor("sinw", [D, CROWS], bf16, **kind).ap(),
        nc.dram_tensor("tsel2", [D, NWT], f32, **kind).ap(),
        nc.dram_tensor("maskl", [128, 128], bf16, **kind).ap(),
        nc.dram_tensor("maskd", [128, 128], bf16, **kind).ap(),
        nc.dram_tensor("idb", [128, 128], bf16, **kind).ap(),
        nc.dram_tensor("out", [NQT, 128, H], f32, **okind).ap(),
    ]
    dummies = None
    if timing:
        dummies = (
            nc.dram_tensor("dummy_in", [1, 8], f32, kind="ExternalInput").ap(),
            nc.dram_tensor("dummy_out", [1, 8], f32, kind="ExternalOutput").ap(),
        )
    with tile.TileContext(nc) as tc:
        _trace(tc, aps)
        if dummies is not None:
            with tc.tile_pool(name="dummy", bufs=1) as dp:
                dt_ = dp.tile([1, 8], f32, name="dummy_sb")
                nc.sync.dma_start(out=dt_, in_=dummies[0])
                nc.sync.dma_start(out=dummies[1], in_=dt_)
    nc.compile()
    return nc


def bench_device(iters=50):
    """Marginal per-iteration time of the compute with dummy-sized I/O.

    Includes the fixed axon dispatch floor (~7 ms) but not the big-tensor
    relay transfers; deltas between kernel variants reflect device time.
    """
    if "timing_runner" not in _CACHE:
        tnc = _build(timing=True)
        _CACHE["timing_runner"] = _Runner(tnc)
    r = _CACHE["timing_runner"]
    maps = [{"dummy_in": np.zeros((1, 8), np.float32)} for _ in range(NCORES)]
    return r.bench(maps, iters=iters)


def _host_constants():
    import ml_dtypes

    inv = 1.0 / (ROPE_THETA ** (np.arange(0, D, 2, dtype=np.float64) / D))
    ii = np.arange(128)
    # masks for S^T [j, i] tiles; valid -> 0, invalid -> NEG
    maskl = np.where(ii[:, None] > ii[None, :], 0.0, NEG).astype(ml_dtypes.bfloat16)
    maskd = np.where(ii[:, None] <= ii[None, :], 0.0, NEG).astype(ml_dtypes.bfloat16)
    idb = np.eye(128).astype(ml_dtypes.bfloat16)

    cos_list, sin_list, tsel_list = [], [], []
    for c in range(NCORES):
        # this core computes K/V for rows [512c-1024, 512c-512) ++
        # [512c, 512c+512)
        pos = np.concatenate([
            np.arange(c * QROWS - WIN, c * QROWS - QROWS, dtype=np.float64),
            np.arange(c * QROWS, c * QROWS + QROWS, dtype=np.float64),
        ])
        pos = np.maximum(pos, 0.0)         # pad rows: value irrelevant (masked)
        ang = inv[:, None] * pos[None, :]  # [64, CROWS]
        cos_list.append(np.concatenate([np.cos(ang), np.cos(ang)], 0)
                        .astype(ml_dtypes.bfloat16))
        sin_list.append(np.concatenate([-np.sin(ang), np.sin(ang)], 0)
                        .astype(ml_dtypes.bfloat16))
        # tsel2[kt] = 0 if local window k-tile kt is a real tile else NEG
        ts = np.zeros((NWT,), np.float32)
        for kt in range(NWT):
            if (c * QROWS - WIN) // 128 + kt < 0:
                ts[kt] = NEG
        tsel_list.append(np.broadcast_to(ts, (128, NWT)).copy())
    return cos_list, sin_list, tsel_list, maskl, maskd, idb


def _get_state():
    if "nc" not in _CACHE:
        _CACHE["nc"] = _build()
        _CACHE["consts"] = _host_constants()
    return _CACHE["nc"], _CACHE["consts"]


def _in_maps(hidden_states, wq, wk, wv, wo, consts):
    import ml_dtypes

    bf16 = ml_dtypes.bfloat16
    hs = np.asarray(hidden_states, np.float32).reshape(L, H).astype(bf16)

    def w_rearrange(w):
        # [kt*128+p, h*128+f] -> [p, h, kt, f] so per-head loads are
        # contiguous 4KB-per-partition DMA lines
        w = np.asarray(w, np.float32).astype(bf16)
        return np.ascontiguousarray(
            w.reshape(16, 128, NH, 128).transpose(1, 2, 0, 3))

    wq = w_rearrange(wq)
    wk = w_rearrange(wk)
    wv = w_rearrange(wv)
    wo = np.asarray(wo, np.float32).astype(bf16)
    cos_list, sin_list, tsel_list, maskl, maskd, idb = consts
    maps = []
    for c in range(NCORES):
        hsw = np.zeros((CROWS, H), bf16)
        lo = c * QROWS - WIN          # chunk0 = rows [lo, hi)
        hi = c * QROWS - QROWS
        src_lo = max(lo, 0)
        if hi > src_lo:
            hsw[src_lo - lo:src_lo - lo + hi - src_lo] = hs[src_lo:hi]
        hsw[QROWS:] = hs[c * QROWS:c * QROWS + QROWS]
        maps.append({
            "hswT": np.ascontiguousarray(hsw.T),
            "wq": wq,
            "wk": wk,
            "wv": wv,
            "wo": wo,
            "cosw": cos_list[c],
            "sinw": sin_list[c],
            "tsel2": tsel_list[c],
            "maskl": maskl,
            "maskd": maskd,
            "idb": idb,
        })
    return maps


def _gather(results):
    full = np.empty((L, H), np.float32)
    for c in range(NCORES):
        full[c * QROWS:(c + 1) * QROWS] = results[c]["out"].reshape(QROWS, H)
    return full.reshape(1, L, H)


class _Runner:
    """Persistent jitted shard_map executable over the 8 axon cores.

    Mirrors bass2jax.run_bass_via_pjrt's multi-core path, but builds the
    jitted callable once (so repeat kernel() calls skip retracing) and
    skips output-buffer donation (this kernel writes every output element,
    so the pre-zeroed-output contract is not needed).
    """

    def __init__(self, nc):
        import jax
        from jax.sharding import Mesh, PartitionSpec
        from jax.experimental.shard_map import shard_map
        from concourse import mybir
        from concourse import bass2jax

        bass2jax.install_neuronx_cc_hook()

        partition_name = (
            nc.partition_id_tensor.name if nc.partition_id_tensor else None
        )
        in_names, out_names, out_avals, zero_outs = [], [], [], []
        for alloc in nc.m.functions[0].allocations:
            if not isinstance(alloc, mybir.MemoryLocationSet):
                continue
            name = alloc.memorylocations[0].name
            if alloc.kind == "ExternalInput":
                if name != partition_name:
                    in_names.append(name)
            elif alloc.kind == "ExternalOutput":
                out_names.append(name)
                shape = tuple(alloc.tensor_shape)
                dtype = mybir.dt.np(alloc.dtype)
                out_avals.append(jax.core.ShapedArray(shape, dtype))
                zero_outs.append(np.zeros(shape, dtype))
        self.n_params = len(in_names)
        self.in_names = list(in_names)
        self.out_names = out_names
        all_names = in_names + out_names
        if partition_name is not None:
            all_names = all_names + [partition_name]

        def _body(*args):
            operands = list(args)
            if partition_name is not None:
                operands.append(bass2jax.partition_id_tensor())
            outs = bass2jax._bass_exec_p.bind(
                *operands,
                out_avals=tuple(out_avals),
                in_names=tuple(all_names),
                out_names=tuple(out_names),
                lowering_input_output_aliases=(),
                sim_require_finite=True,
                sim_require_nnan=True,
                nc=nc,
            )
            return tuple(outs)

        devices = jax.devices()[:NCORES]
        assert len(devices) == NCORES
        self.mesh = Mesh(np.asarray(devices), ("core",))
        in_specs = (PartitionSpec("core"),) * (self.n_params + len(out_names))
        out_specs = (PartitionSpec("core"),) * len(out_names)
        self.sharded = jax.jit(
            shard_map(_body, mesh=self.mesh, in_specs=in_specs,
                      out_specs=out_specs, check_rep=False),
            keep_unused=True,
        )
        self.out_avals = out_avals
        self.concat_zeros = [
            np.zeros((NCORES * z.shape[0], *z.shape[1:]), z.dtype)
            for z in zero_outs
        ]
        self._dev_args = None

    def pack(self, maps):
        return [
            np.concatenate([np.asarray(maps[c][n]) for c in range(NCORES)], axis=0)
            for n in self.in_names
        ]

    def run(self, maps):
        import jax

        concat_in = self.pack(maps)
        out_arrs = self.sharded(*concat_in, *self.concat_zeros)
        return [
            {
                n: np.asarray(out_arrs[i]).reshape(
                    NCORES, *self.out_avals[i].shape)[c]
                for i, n in enumerate(self.out_names)
            }
            for c in range(NCORES)
        ]

    def bench(self, maps, iters=10):
        """Time repeated executions with inputs resident on device."""
        import time

        import jax

        args = [jax.device_put(a) for a in self.pack(maps)]
        args += [jax.device_put(z) for z in self.concat_zeros]
        out = self.sharded(*args)  # warm
        jax.block_until_ready(out)
        t0 = time.perf_counter()
        for _ in range(iters):
            out = self.sharded(*args)
        jax.block_until_ready(out)
        return (time.perf_counter() - t0) / iters


def _get_runner():
    nc, consts = _get_state()
    if "runner" not in _CACHE:
        _CACHE["runner"] = _Runner(nc)
    return _CACHE["runner"], consts


def kernel(hidden_states, wq, wk, wv, wo):
    runner, consts = _get_runner()
    maps = _in_maps(hidden_states, wq, wk, wv, wo, consts)
    return _gather(runner.run(maps))


def bench(hidden_states, wq, wk, wv, wo, iters=10):
    runner, consts = _get_runner()
    maps = _in_maps(hidden_states, wq, wk, wv, wo, consts)
    return runner.bench(maps, iters=iters)

